# revision 1
# baseline (speedup 1.0000x reference)
"""Directed multi-head attention (sparse top-64) TRN2 Bass kernel.

Strategy: data-parallel over batch (16 batches -> 2 per core on 8 cores).
Per core, per batch:
  - projections compute qT/kT in [channel, token] layout and v in
    [token, channel] layout via PE matmuls (weights pre-transposed on host,
    1/sqrt(Dh) folded into Wq, head_gates folded into Wo)
  - per head: scores = qT.T @ kT accumulated with the relative-position bias
    (added via an identity-matmul accumulation into the same PSUM bank);
    per-row top-64 threshold extracted with 8 rounds of DVE max8 +
    match_replace; softmax folded as exp(s - m - ln Z) with Z from the
    extracted top values; mask+multiply fused in one scalar_tensor_tensor;
    PE transposes p, then AV matmul produces avT[channel, token]
  - output projection from avT, result written transposed; host untransposes.
"""

import sys

sys.path.insert(0, "/opt/trn_rl_repo")

import numpy as np

import concourse.bass as bass
import concourse.tile as tile
import concourse.mybir as mybir
from concourse.bass_utils import run_bass_kernel_spmd

F32 = mybir.dt.float32
AF = mybir.ActivationFunctionType
OP = mybir.AluOpType

B, T, C = 16, 512, 1024
H, DH = 16, 64
TOPK = 64
MAXPOS = 512
NCORES = 8
BPC = B // NCORES  # batches per core
NEG = -3.0e38


def _legalize_waits(nc):
    """This walrus build rejects CTRL-encoded instructions (Drain/NoOp) with
    more than one sem-wait. Hoist excess waits onto InstEventSemaphore
    carriers (one wait each) inserted just before, same engine."""
    n_fix = 0
    for f in nc.m.functions:
        for bb in f.blocks:
            insts = bb.instructions
            out = []
            changed = False
            for inst in insts:
                si = inst.sync_info
                waits = list(si.on_wait) if si is not None else []
                limit = 1
                if len(waits) > limit:
                    extra, keep = waits[:-limit], waits[-limit:]
                    for k, w in enumerate(extra):
                        ev = mybir.InstEventSemaphore(name=f"{inst.name}-hoistw{k}")
                        ev.engine = inst.engine
                        ev.sync_info = mybir.SyncInfo(on_wait=[w], on_update=[])
                        out.append(ev)
                    inst.sync_info = mybir.SyncInfo(
                        on_wait=keep, on_update=list(si.on_update)
                    )
                    n_fix += 1
                    changed = True
                out.append(inst)
            if changed:
                bb.instructions = out
    return n_fix


def _build_program():
    nc = bass.Bass(trn_type="TRN2", target_bir_lowering=False, debug=False)

    xt_d = nc.dram_tensor("xt", [BPC, C, T], F32, kind="ExternalInput").ap()
    wq_d = nc.dram_tensor("wq", [C, C], F32, kind="ExternalInput").ap()
    wk_d = nc.dram_tensor("wk", [C, C], F32, kind="ExternalInput").ap()
    wv_d = nc.dram_tensor("wv", [C, C], F32, kind="ExternalInput").ap()
    wo_d = nc.dram_tensor("wo", [C, C], F32, kind="ExternalInput").ap()
    bq_d = nc.dram_tensor("bq", [8, 128, 1], F32, kind="ExternalInput").ap()
    bk_d = nc.dram_tensor("bk", [8, 128, 1], F32, kind="ExternalInput").ap()
    bv_d = nc.dram_tensor("bv", [1, C], F32, kind="ExternalInput").ap()
    bo_d = nc.dram_tensor("bo", [8, 128, 1], F32, kind="ExternalInput").ap()
    bias_d = nc.dram_tensor("biastab", [H, T, T], F32, kind="ExternalInput").ap()
    id_d = nc.dram_tensor("ident", [128, 128], F32, kind="ExternalInput").ap()
    yt_d = nc.dram_tensor("yt", [BPC, C, T], F32, kind="ExternalOutput").ap()

    with tile.TileContext(nc) as tc:
        from contextlib import ExitStack

        with ExitStack() as ctx:
            ep_ = ctx.enter_context

            # constants
            const_p = ep_(tc.tile_pool(name="const", bufs=1))
            ident = const_p.tile([128, 128], F32, tag="ident", name="ident")
            nc.sync.dma_start(ident[:], id_d[:])
            ones_row = const_p.tile([1, 128], F32, tag="ones", name="ones")
            nc.gpsimd.memset(ones_row[:], 1.0)
            bq_sb = const_p.tile([128, 8], F32, tag="bq", name="bq")
            bk_sb = const_p.tile([128, 8], F32, tag="bk", name="bk")
            bo_sb = const_p.tile([128, 8], F32, tag="bo", name="bo")
            bv_sb = const_p.tile([1, C], F32, tag="bv", name="bv")
            for j in range(8):
                nc.sync.dma_start(bq_sb[:, j : j + 1], bq_d[j])
                nc.sync.dma_start(bk_sb[:, j : j + 1], bk_d[j])
                nc.sync.dma_start(bo_sb[:, j : j + 1], bo_d[j])
            nc.sync.dma_start(bv_sb[:], bv_d[:])

            # persistent per-batch tensors (8 x [128,512] each)
            xt_p = ep_(tc.tile_pool(name="xt", bufs=1))
            qt_p = ep_(tc.tile_pool(name="qt", bufs=1))
            kt_p = ep_(tc.tile_pool(name="kt", bufs=1))
            v_p = ep_(tc.tile_pool(name="v", bufs=1))
            avt_p = ep_(tc.tile_pool(name="avt", bufs=1))

            # streamed weights
            w_p = ep_(tc.tile_pool(name="w", bufs=6))
            wv_p = ep_(tc.tile_pool(name="wv", bufs=4))
            biasp = ep_(tc.tile_pool(name="bias", bufs=8))

            # attention working tiles
            ssb_p = ep_(tc.tile_pool(name="ssb", bufs=6))
            scr_p = ep_(tc.tile_pool(name="scr", bufs=6))
            tv_p = ep_(tc.tile_pool(name="tv", bufs=6))
            ep_p = ep_(tc.tile_pool(name="ep", bufs=5))
            p_p = ep_(tc.tile_pool(name="p", bufs=5))
            pt_p = ep_(tc.tile_pool(name="pt", bufs=5))
            sm_p = ep_(tc.tile_pool(name="small", bufs=8))
            e8_p = ep_(tc.tile_pool(name="e8", bufs=4))
            y_p = ep_(tc.tile_pool(name="ysb", bufs=2))

            # PSUM pools (8 banks total)
            prps = ep_(tc.tile_pool(name="prps", bufs=2, space="PSUM"))
            scps = ep_(tc.tile_pool(name="scps", bufs=2, space="PSUM"))
            ptps = ep_(tc.tile_pool(name="ptps", bufs=2, space="PSUM"))
            avps = ep_(tc.tile_pool(name="avps", bufs=2, space="PSUM"))

            for b in range(BPC):
                # ---- load xT for this batch ----
                xts = []
                for ct in range(8):
                    t_ = xt_p.tile([128, T], F32, tag=f"xt{ct}", name=f"xt{ct}")
                    nc.sync.dma_start(t_[:], xt_d[b, ct * 128 : (ct + 1) * 128, :])
                    xts.append(t_)

                # ---- q/k projections -> qT,kT [o,t] ----
                qts, kts = [], []
                for name, wd, bsb, outlist, pool in (
                    ("q", wq_d, bq_sb, qts, qt_p),
                    ("k", wk_d, bk_sb, kts, kt_p),
                ):
                    for ot in range(8):
                        ps = prps.tile([128, T], F32, tag="proj", name="proj")
                        for ct in range(8):
                            w = w_p.tile([128, 128], F32, tag="w", name="w")
                            nc.sync.dma_start(
                                w[:],
                                wd[ct * 128 : (ct + 1) * 128, ot * 128 : (ot + 1) * 128],
                            )
                            nc.tensor.matmul(
                                ps[:], w[:], xts[ct][:], start=(ct == 0), stop=(ct == 7)
                            )
                        dst = pool.tile([128, T], F32, tag=f"{name}{ot}", name=f"{name}{ot}")
                        nc.scalar.activation(
                            dst[:], ps[:], AF.Identity, bias=bsb[:, ot : ot + 1]
                        )
                        outlist.append(dst)

                # ---- v projection -> v [t,o] natural ----
                vts = [[None] * 2 for _ in range(4)]
                for tt in range(4):
                    for oc in range(2):
                        ps = prps.tile([128, T], F32, tag="proj", name="proj")
                        for ct in range(8):
                            wv = wv_p.tile([128, 512], F32, tag="wv", name="wv")
                            nc.sync.dma_start(
                                wv[:],
                                wv_d[ct * 128 : (ct + 1) * 128, oc * 512 : (oc + 1) * 512],
                            )
                            nc.tensor.matmul(
                                ps[:],
                                xts[ct][:, tt * 128 : (tt + 1) * 128],
                                wv[:],
                                start=(ct == 0),
                                stop=False,
                            )
                        # bias rank-1: out[t,o] += 1 * bv[o]
                        nc.tensor.matmul(
                            ps[:],
                            ones_row[:],
                            bv_sb[:, oc * 512 : (oc + 1) * 512],
                            start=False,
                            stop=True,
                        )
                        dst = v_p.tile([128, 512], F32, tag=f"v{tt}_{oc}", name=f"v{tt}_{oc}")
                        nc.scalar.activation(dst[:], ps[:], AF.Copy)
                        vts[tt][oc] = dst

                # ---- attention per head ----
                avts = [avt_p.tile([128, T], F32, tag=f"avt{j}", name=f"avt{j}") for j in range(8)]
                for h in range(16):
                    qh = qts[h // 2][(h % 2) * 64 : (h % 2) * 64 + 64, :]
                    kh = kts[h // 2][(h % 2) * 64 : (h % 2) * 64 + 64, :]

                    btiles = []
                    for tt in range(4):
                        bt = biasp.tile([128, T], F32, tag="bias", name="bias")
                        nc.sync.dma_start(
                            bt[:], bias_d[h, tt * 128 : (tt + 1) * 128, :]
                        )
                        btiles.append(bt)

                    ssbs, scrs, tvs = [], [], []
                    for tt in range(4):
                        ps = scps.tile([128, T], F32, tag="sc", name="sc")
                        nc.tensor.matmul(
                            ps[:],
                            qh[:, tt * 128 : (tt + 1) * 128],
                            kh,
                            start=True,
                            stop=False,
                        )
                        nc.tensor.matmul(
                            ps[:], ident[:], btiles[tt][:], start=False, stop=True
                        )
                        ssb = ssb_p.tile([128, T], F32, tag="ssb", name="ssb")
                        nc.scalar.activation(ssb[:], ps[:], AF.Copy)
                        scr = scr_p.tile([128, T], F32, tag="scr", name="scr")
                        nc.scalar.activation(scr[:], ps[:], AF.Copy)
                        ssbs.append(ssb)
                        scrs.append(scr)
                        tvs.append(tv_p.tile([128, TOPK], F32, tag="tv", name="tv"))

                    # top-64 extraction, rounds interleaved across the 4 tiles
                    for r in range(8):
                        for tt in range(4):
                            nc.vector.max(tvs[tt][:, r * 8 : r * 8 + 8], scrs[tt][:])
                            if r < 7:
                                nc.vector.match_replace(
                                    scrs[tt][:],
                                    tvs[tt][:, r * 8 : r * 8 + 8],
                                    scrs[tt][:],
                                    NEG,
                                )

                    ptiles = []
                    for tt in range(4):
                        tv = tvs[tt]
                        negm = sm_p.tile([128, 1], F32, tag="negm", name="negm")
                        nc.scalar.activation(negm[:], tv[:, 0:1], AF.Copy, scale=-1.0)
                        e8 = e8_p.tile([128, TOPK], F32, tag="e8", name="e8")
                        z = sm_p.tile([128, 1], F32, tag="z", name="z")
                        nc.scalar.activation(
                            e8[:], tv[:], AF.Exp, bias=negm[:], accum_out=z[:]
                        )
                        b2 = sm_p.tile([128, 1], F32, tag="b2", name="b2")
                        nc.scalar.activation(b2[:], z[:], AF.Ln)
                        b3 = sm_p.tile([128, 1], F32, tag="b3", name="b3")
                        nc.scalar.activation(
                            b3[:], b2[:], AF.Identity, scale=-1.0, bias=negm[:]
                        )
                        ept = ep_p.tile([128, T], F32, tag="ep", name="ep")
                        nc.scalar.activation(ept[:], ssbs[tt][:], AF.Exp, bias=b3[:])
                        pt = p_p.tile([128, T], F32, tag="p", name="p")
                        nc.vector.scalar_tensor_tensor(
                            pt[:],
                            ssbs[tt][:],
                            tv[:, 63:64],
                            ept[:],
                            op0=OP.is_ge,
                            op1=OP.mult,
                        )
                        ptiles.append(pt)

                    # transpose p -> pT, per s-tile
                    pts = []
                    for st in range(4):
                        pps = ptps.tile([128, T], F32, tag="ptps", name="ptps")
                        for tt in range(4):
                            nc.tensor.transpose(
                                pps[:, tt * 128 : (tt + 1) * 128],
                                ptiles[tt][:, st * 128 : (st + 1) * 128],
                                ident[:],
                            )
                        psb = pt_p.tile([128, T], F32, tag="pt", name="pt")
                        nc.scalar.activation(psb[:], pps[:], AF.Copy)
                        pts.append(psb)

                    # AV: avT[d, t] accumulated over s tiles
                    avp = avps.tile([64, T], F32, tag="av", name="av")
                    for st in range(4):
                        vh = vts[st][h // 8][:, (h % 8) * 64 : (h % 8) * 64 + 64]
                        nc.tensor.matmul(
                            avp[:], vh, pts[st][:], start=(st == 0), stop=(st == 3)
                        )
                    off = (h % 2) * 64
                    nc.scalar.activation(
                        avts[h // 2][off : off + 64, :], avp[:], AF.Copy
                    )

                # ---- output projection yT[e,t] ----
                for et in range(8):
                    ps = prps.tile([128, T], F32, tag="proj", name="proj")
                    for ot in range(8):
                        w = w_p.tile([128, 128], F32, tag="w", name="w")
                        nc.sync.dma_start(
                            w[:],
                            wo_d[ot * 128 : (ot + 1) * 128, et * 128 : (et + 1) * 128],
                        )
                        nc.tensor.matmul(
                            ps[:], w[:], avts[ot][:], start=(ot == 0), stop=(ot == 7)
                        )
                    ysb = y_p.tile([128, T], F32, tag="ysb", name="ysb")
                    nc.scalar.activation(
                        ysb[:], ps[:], AF.Identity, bias=bo_sb[:, et : et + 1]
                    )
                    nc.sync.dma_start(yt_d[b, et * 128 : (et + 1) * 128, :], ysb[:])

    _legalize_waits(nc)
    return nc


_CACHED_NC = None


def _get_nc():
    global _CACHED_NC
    if _CACHED_NC is None:
        _CACHED_NC = _build_program()
    return _CACHED_NC


def _host_prep(inputs):
    x = np.asarray(inputs["x"], dtype=np.float32)
    Wq = np.asarray(inputs["Wq"], dtype=np.float32)
    bq = np.asarray(inputs["bq"], dtype=np.float32)
    Wk = np.asarray(inputs["Wk"], dtype=np.float32)
    bk = np.asarray(inputs["bk"], dtype=np.float32)
    Wv = np.asarray(inputs["Wv"], dtype=np.float32)
    bv = np.asarray(inputs["bv"], dtype=np.float32)
    Wo = np.asarray(inputs["Wo"], dtype=np.float32)
    bo = np.asarray(inputs["bo"], dtype=np.float32)
    gates = np.asarray(inputs["head_gates"], dtype=np.float32)
    rpb = np.asarray(inputs["rel_pos_bias"], dtype=np.float32)

    shared = {
        "wq": np.ascontiguousarray((Wq / 8.0).T),
        "wk": np.ascontiguousarray(Wk.T),
        "wv": np.ascontiguousarray(Wv.T),
        "wo": np.ascontiguousarray((Wo * np.repeat(gates, DH)[None, :]).T),
        "bq": np.ascontiguousarray((bq / 8.0).reshape(8, 128, 1)),
        "bk": np.ascontiguousarray(bk.reshape(8, 128, 1)),
        "bv": np.ascontiguousarray(bv.reshape(1, C)),
        "bo": np.ascontiguousarray(bo.reshape(8, 128, 1)),
        "ident": np.eye(128, dtype=np.float32),
    }
    r = np.arange(T)
    idx = r[None, :] - r[:, None] + (MAXPOS - 1)  # [T,T]
    shared["biastab"] = np.ascontiguousarray(rpb[idx].transpose(2, 0, 1))  # [H,T,T]

    in_maps = []
    for core in range(NCORES):
        xs = x[core * BPC : (core + 1) * BPC]  # [2,T,C]
        m = dict(shared)
        m["xt"] = np.ascontiguousarray(xs.transpose(0, 2, 1))  # [2,C,T]
        in_maps.append(m)
    return in_maps


def _gather(results):
    out = np.empty((B, T, C), dtype=np.float32)
    for core in range(NCORES):
        yt = results[core]["yt"]  # [2, C, T]
        for b in range(BPC):
            out[core * BPC + b] = yt[b].T
    return out


def kernel(**inputs) -> np.ndarray:
    nc = _get_nc()
    in_maps = _host_prep(inputs)
    res = run_bass_kernel_spmd(nc, in_maps, list(range(NCORES)))
    return _gather(res.results)

